# revision 10
# baseline (speedup 1.0000x reference)
"""Trainium2 Bass kernel for nn_Mnist_lmdSplineKAN.

Sharding: data-parallel over batch, 8 cores x 128 rows. All params replicated.

Per-core math (I=784 inputs, H=10 heads, O=64, 8 B-spline basis fns, order 3,
5 uniform intervals on [0,1)):
  t = floor(5x) (int-round trick), u = 5x - t, one-hot masks m_t = (t == const)
  features[b,i,j] = sum_t m_t * p_{j-t}(u)  with p = 6x local cubic polys
  features[b,i,8] = silu(x[b,i])
  y[b,(h,o)] = sum_{i,j} features[b,i,j] * Wbig[(i,j),(h,o)]  (fp16 matmul;
               Wbig folds coef*scale_sp*lmd/6 and scale_base*lmd)
  h1 = tanh(y); h2 = tanh(h1 @ blockdiag(W1) + b1); logits = <h2,W2>_head + b2
"""
import sys, types
import numpy as np

B, I, O, H, NB = 1024, 784, 64, 10, 8
NC = 8
BC = B // NC      # 128
P = 112
CH = I // P       # 7
HO = H * O        # 640
D2 = H * 32       # 320
NH = 2            # PSUM halves of HO
JSPLIT = 5        # weight dma piece A covers j<JSPLIT, piece B the rest


def _install_ntff_hook():
    if "antenv.axon_hooks" in sys.modules:
        return
    try:
        import antenv
        mod = types.ModuleType("antenv.axon_hooks")
        _h = [None]
        mod.set_axon_ntff_profile_hook = lambda h: _h.__setitem__(0, h)
        mod.get_axon_ntff_profile_hook = lambda: _h[0]
        sys.modules["antenv.axon_hooks"] = mod
        antenv.axon_hooks = mod
        from trn_agent_boot.trn_boot import _ntff_profile_via_ctypes
        h = _ntff_profile_via_ctypes("/opt/axon/libaxon_pjrt.so")
        if h is not None:
            mod.set_axon_ntff_profile_hook(h)
    except Exception:
        pass


_CACHE = {}


def _build():
    if "nc" in _CACHE:
        return _CACHE["nc"]
    import concourse.bacc as bacc
    import concourse.bass as bass
    import concourse.tile as tile
    from concourse import mybir
    from contextlib import ExitStack

    f32, f16, i32 = mybir.dt.float32, mybir.dt.float16, mybir.dt.int32
    ALU = mybir.AluOpType
    AF = mybir.ActivationFunctionType

    nc = bacc.Bacc("TRN2", target_bir_lowering=False, debug=False)
    x_d = nc.dram_tensor("x", (P, CH, BC), f32, kind="ExternalInput").ap()
    w_d = nc.dram_tensor("w", (P * CH * (NB + 1) * HO,), f16, kind="ExternalInput").ap()
    w1_d = nc.dram_tensor("w1", (128, 5 * D2 + 128), f16, kind="ExternalInput").ap()
    b1_d = nc.dram_tensor("b1", (1, D2), f16, kind="ExternalInput").ap()
    w2_d = nc.dram_tensor("w2", (128, D2 + H), f32, kind="ExternalInput").ap()
    out_d = nc.dram_tensor("out", (BC, H), f32, kind="ExternalOutput").ap()

    def bcast(dram_ap, n):
        return bass.AP(tensor=dram_ap.tensor, offset=dram_ap.offset,
                       ap=[[0, 128]] + [[1, n]])

    with tile.TileContext(nc) as tc, ExitStack() as ctx:
        sb = ctx.enter_context(tc.tile_pool(name="sb", bufs=1))
        ps = ctx.enter_context(tc.tile_pool(name="ps", bufs=1, space="PSUM"))

        # ---- x split across both HWDGE queues: lands first ----
        xt = sb.tile([P, CH, BC], f32, tag="xt")
        nc.sync.dma_start(xt[:, 0:4, :], x_d[:, 0:4, :])
        nc.scalar.dma_start(xt[:, 4:CH, :], x_d[:, 4:CH, :])
        ones = sb.tile([1, 128], f16, tag="ones")
        nc.vector.memset(ones[:], 1.0)

        # ---- weights: p-major layout; pieces of whole chunks with multi-chunk
        #      contiguous per-partition runs (big descriptors), all on the SP
        #      queue in consumption order ----
        CGRP = [(0, 1), (1, 2), (2, 3), (3, 4), (4, 5), (5, 6), (6, 7)]
        ROW = (NB + 1) * HO  # elems per (p, c)
        wg = []
        off = 0
        for g, (c0, c1) in enumerate(CGRP):
            t = sb.tile([P, c1 - c0, NB + 1, HO], f16, tag=f"wg{g}",
                        name=f"wg{g}")
            run = (c1 - c0) * ROW
            src = bass.AP(tensor=w_d.tensor, offset=off,
                          ap=[[run, P], [1, run]])
            (nc.gpsimd if g % 2 == 0 else nc.sync).dma_start(t[:], src)
            wg.append(t)
            off += P * run

        def wslice(c, j, nh):
            for g, (c0, c1) in enumerate(CGRP):
                if c0 <= c < c1:
                    return wg[g][:, c - c0, j, nh * D2:(nh + 1) * D2]

        c16 = sb.tile([128, 5 * D2 + 128], f16, tag="c16")
        nc.scalar.dma_start(c16[:], w1_d)
        w1t = c16[:, 0:5 * D2].rearrange("p (k d) -> p k d", d=D2)
        idt = c16[:, 5 * D2:]
        c32 = sb.tile([128, D2 + H], f32, tag="c32")
        nc.scalar.dma_start(c32[:], w2_d)
        w2b = c32[:, 0:D2]
        b2b = c32[:, D2:]
        b1r = sb.tile([1, D2], f16, tag="b1r")
        nc.scalar.dma_start(b1r[:], b1_d)

        x = xt[:].rearrange("p c b -> p (c b)")

        def T(tag, dt=f16):
            return sb.tile([P, CH * BC], dt, tag=tag, name=tag)

        # ---- interval index t = floor(5x) via round(5x-0.5); masks; u ----
        ti = T("ti", i32)
        nc.vector.tensor_scalar(ti[:], x, 5.0, -0.5, op0=ALU.mult, op1=ALU.add)
        m = []
        for t in range(5):
            mt = T(f"m{t}")
            nc.vector.tensor_scalar(mt[:], ti[:], t, None, op0=ALU.is_equal)
            m.append(mt)
        tf = T("tf", f32)
        nc.vector.tensor_copy(tf[:], ti[:])
        u = T("u", f32)
        nc.vector.scalar_tensor_tensor(u[:], x, 5.0, tf[:],
                                       op0=ALU.mult, op1=ALU.subtract)

        # ---- local cubics (x6): p0=(1-u)^3, p1=3u^3-6u^2+4=(3u-6)u^2+4,
        #      p2=p1(1-u), p3=u^3 ----
        u2 = T("u2", f32); nc.scalar.activation(u2[:], u[:], AF.Square)
        w_ = T("w_", f32)
        nc.scalar.activation(w_[:], u[:], AF.Copy, bias=1.0, scale=-1.0)
        w2_ = T("w2_", f32); nc.scalar.activation(w2_[:], w_[:], AF.Square)
        a_ = T("a_", f32)
        nc.scalar.activation(a_[:], u[:], AF.Copy, bias=-6.0, scale=3.0)
        b_ = T("b_", f32)
        nc.scalar.activation(b_[:], w_[:], AF.Copy, bias=-6.0, scale=3.0)
        p3h = T("p3h"); nc.vector.tensor_tensor(p3h[:], u2[:], u[:], op=ALU.mult)
        p0h = T("p0h"); nc.vector.tensor_tensor(p0h[:], w2_[:], w_[:], op=ALU.mult)
        p1pre = T("p1pre", f32)
        nc.vector.tensor_tensor(p1pre[:], a_[:], u2[:], op=ALU.mult)
        p1h = T("p1h")
        nc.scalar.activation(p1h[:], p1pre[:], AF.Copy, bias=4.0, scale=1.0)
        p2pre = T("p2pre", f32)
        nc.vector.tensor_tensor(p2pre[:], b_[:], w2_[:], op=ALU.mult)
        p2h = T("p2h")
        nc.scalar.activation(p2h[:], p2pre[:], AF.Copy, bias=4.0, scale=1.0)
        ph = [p0h, p1h, p2h, p3h]

        # ---- features ----
        f_ = []
        for j in range(NB):
            f_.append(sb.tile([P, CH, BC], f16, tag=f"f{j}", name=f"f{j}"))
        fs = sb.tile([P, CH, BC], f16, tag="f8")
        nc.scalar.activation(fs[:].rearrange("p c b -> p (c b)"), x, AF.Silu)
        f_.append(fs)

        psum = [ps.tile([128, D2], f32, tag=f"y{nh}", name=f"y{nh}")
                for nh in range(NH)]

        tmp = T("tmp")
        tmp2 = T("tmp2")
        for j in range(NB):
            terms = [(t, j - t) for t in range(5) if 0 <= j - t <= 3]
            out = f_[j][:].rearrange("p c b -> p (c b)")
            if len(terms) == 1:
                t, r = terms[0]
                nc.vector.tensor_tensor(out, m[t][:], ph[r][:], op=ALU.mult)
            else:
                acc = tmp[:]
                t, r = terms[0]
                nc.vector.tensor_tensor(acc, m[t][:], ph[r][:], op=ALU.mult)
                for k, (t, r) in enumerate(terms[1:]):
                    pr = tmp2[:]
                    nc.vector.tensor_tensor(pr, m[t][:], ph[r][:], op=ALU.mult)
                    dst = out if k == len(terms) - 2 else acc
                    nc.vector.tensor_tensor(dst, acc, pr, op=ALU.add)

        nmm = 0
        NTOT = CH * (NB + 1)
        for g, (c0, c1) in enumerate(CGRP):
            for j in range(NB + 1):
                for c in range(c0, c1):
                    for nh in range(NH):
                        nc.tensor.matmul(
                            psum[nh][:],
                            f_[j][:, c, :],
                            wslice(c, j, nh),
                            start=(nmm == 0),
                            stop=(nmm == NTOT - 1),
                        )
                    nmm += 1

        # ---- tail ----
        h1 = sb.tile([128, HO], f16, tag="h1")
        for nh in range(NH):
            nc.scalar.activation(h1[:, nh * D2:(nh + 1) * D2], psum[nh][:], AF.Tanh)
        h1t = []
        for k in range(5):
            pt = ps.tile([128, 128], f16, tag=f"pt{k}", name=f"pt{k}")
            nc.tensor.transpose(pt[:], h1[:, k * 128:(k + 1) * 128], idt)
            st = sb.tile([128, 128], f16, tag=f"h1t{k}", name=f"h1t{k}")
            nc.vector.tensor_copy(st[:], pt[:])
            h1t.append(st)
        ps2 = ps.tile([128, D2], f32, tag="ps2")
        for k in range(5):
            nc.tensor.matmul(ps2[:], h1t[k][:], w1t[:, k, :],
                             start=(k == 0), stop=False)
        nc.tensor.matmul(ps2[:], ones[:], b1r[:], start=False, stop=True)
        h2 = sb.tile([128, D2], f32, tag="h2")
        nc.scalar.activation(h2[:], ps2[:], AF.Tanh)
        prod = sb.tile([128, D2], f32, tag="prod")
        nc.vector.tensor_tensor(prod[:], h2[:], w2b, op=ALU.mult)
        red = sb.tile([128, H], f32, tag="red")
        nc.vector.tensor_reduce(red[:], prod[:].rearrange("p (h d) -> p h d", d=32),
                                axis=mybir.AxisListType.X, op=ALU.add)
        lg = sb.tile([128, H], f32, tag="lg")
        nc.vector.tensor_tensor(lg[:], red[:], b2b, op=ALU.add)
        nc.sync.dma_start(out_d, lg[:])

    nc.compile()
    _CACHE["nc"] = nc
    return nc


def _prep_inputs(x, coef, scale_base, scale_sp, lmd, W1, b1, W2, b2):
    xf = np.asarray(x, np.float64).reshape(B, I)
    coef = np.asarray(coef, np.float64)
    eff = coef * np.asarray(scale_sp, np.float64)[..., None] \
        * np.asarray(lmd, np.float64)[:, :, None, None] / 6.0
    sbl = np.asarray(scale_base, np.float64) \
        * np.asarray(lmd, np.float64)[:, :, None]
    wbig = np.concatenate([eff, sbl[..., None]], -1)               # (H,I,O,9)
    wp = wbig.reshape(H, CH, P, O, NB + 1).transpose(2, 1, 4, 0, 3) \
        .astype(np.float16)                                        # (P,CH,9,HO)
    wdev = np.ascontiguousarray(wp.transpose(1, 0, 2, 3, 4)).reshape(-1)  # piece-major (per chunk)
    W1 = np.asarray(W1, np.float64)
    w1bd = np.zeros((HO, D2))
    for h in range(H):
        w1bd[h * O:(h + 1) * O, h * 32:(h + 1) * 32] = W1[h]
    w1dev = np.ascontiguousarray(
        w1bd.reshape(5, 128, D2).transpose(1, 0, 2)).astype(np.float16)
    c16 = np.concatenate([w1dev.reshape(128, 5 * D2),
                          np.eye(128, dtype=np.float16)], 1).astype(np.float16)
    b1c = np.asarray(b1, np.float16).reshape(1, D2).copy()
    c32 = np.concatenate([
        np.broadcast_to(np.asarray(W2, np.float32).reshape(D2), (128, D2)),
        np.broadcast_to(np.asarray(b2, np.float32).reshape(H), (128, H))],
        1).astype(np.float32)
    c32 = np.ascontiguousarray(c32)

    in_maps = []
    for core in range(NC):
        xs = xf[core * BC:(core + 1) * BC].T
        xdev = np.ascontiguousarray(
            xs.reshape(CH, P, BC).transpose(1, 0, 2)).astype(np.float32)
        in_maps.append({"x": xdev, "w": wdev, "w1": c16,
                        "b1": b1c, "w2": c32})
    return in_maps


def run(inputs, trace=False, tmpdir=None):
    _install_ntff_hook()
    from concourse.bass_utils import run_bass_kernel_spmd
    nc = _build()
    in_maps = _prep_inputs(**inputs)
    res = run_bass_kernel_spmd(nc, in_maps, core_ids=list(range(NC)),
                               trace=trace, tmpdir=tmpdir)
    out = np.concatenate([r["out"] for r in res.results], 0)
    return out.astype(np.float32), res


def kernel(**inputs):
    out, _ = run(inputs)
    return out


# revision 11
# speedup vs baseline: 1.0130x; 1.0130x over previous
"""Trainium2 Bass kernel for nn_Mnist_lmdSplineKAN.

Sharding: data-parallel over batch, 8 cores x 128 rows. All params replicated.

Per-core math (I=784 inputs, H=10 heads, O=64, 8 B-spline basis fns, order 3,
5 uniform intervals on [0,1)):
  t = floor(5x) (int-round trick), u = 5x - t, one-hot masks m_t = (t == const)
  features[b,i,j] = sum_t m_t * p_{j-t}(u)  with p = 6x local cubic polys
  features[b,i,8] = silu(x[b,i])
  y[b,(h,o)] = sum_{i,j} features[b,i,j] * Wbig[(i,j),(h,o)]  (fp16 matmul;
               Wbig folds coef*scale_sp*lmd/6 and scale_base*lmd)
  h1 = tanh(y); h2 = tanh(h1 @ blockdiag(W1) + b1); logits = <h2,W2>_head + b2
"""
import sys, types
import numpy as np

B, I, O, H, NB = 1024, 784, 64, 10, 8
NC = 8
BC = B // NC      # 128
P = 112
CH = I // P       # 7
HO = H * O        # 640
D2 = H * 32       # 320
NH = 2            # PSUM halves of HO
JSPLIT = 5        # weight dma piece A covers j<JSPLIT, piece B the rest


def _install_ntff_hook():
    if "antenv.axon_hooks" in sys.modules:
        return
    try:
        import antenv
        mod = types.ModuleType("antenv.axon_hooks")
        _h = [None]
        mod.set_axon_ntff_profile_hook = lambda h: _h.__setitem__(0, h)
        mod.get_axon_ntff_profile_hook = lambda: _h[0]
        sys.modules["antenv.axon_hooks"] = mod
        antenv.axon_hooks = mod
        from trn_agent_boot.trn_boot import _ntff_profile_via_ctypes
        h = _ntff_profile_via_ctypes("/opt/axon/libaxon_pjrt.so")
        if h is not None:
            mod.set_axon_ntff_profile_hook(h)
    except Exception:
        pass


_CACHE = {}


def _build():
    if "nc" in _CACHE:
        return _CACHE["nc"]
    import concourse.bacc as bacc
    import concourse.bass as bass
    import concourse.tile as tile
    from concourse import mybir
    from contextlib import ExitStack

    f32, f16, i32 = mybir.dt.float32, mybir.dt.float16, mybir.dt.int32
    ALU = mybir.AluOpType
    AF = mybir.ActivationFunctionType

    nc = bacc.Bacc("TRN2", target_bir_lowering=False, debug=False)
    x_d = nc.dram_tensor("x", (P, CH, BC), f32, kind="ExternalInput").ap()
    w_d = nc.dram_tensor("w", (P * CH * (NB + 1) * HO,), f16, kind="ExternalInput").ap()
    w1_d = nc.dram_tensor("w1", (128, 5 * D2 + 128), f16, kind="ExternalInput").ap()
    b1_d = nc.dram_tensor("b1", (1, D2), f16, kind="ExternalInput").ap()
    w2_d = nc.dram_tensor("w2", (128, D2 + H), f32, kind="ExternalInput").ap()
    out_d = nc.dram_tensor("out", (BC, H), f32, kind="ExternalOutput").ap()

    def bcast(dram_ap, n):
        return bass.AP(tensor=dram_ap.tensor, offset=dram_ap.offset,
                       ap=[[0, 128]] + [[1, n]])

    with tile.TileContext(nc) as tc, ExitStack() as ctx:
        sb = ctx.enter_context(tc.tile_pool(name="sb", bufs=1))
        ps = ctx.enter_context(tc.tile_pool(name="ps", bufs=1, space="PSUM"))

        # ---- x split across both HWDGE queues: lands first ----
        xt = sb.tile([P, CH, BC], f32, tag="xt")
        nc.sync.dma_start(xt[:, 0:4, :], x_d[:, 0:4, :])
        nc.scalar.dma_start(xt[:, 4:CH, :], x_d[:, 4:CH, :])
        ones = sb.tile([1, 128], f16, tag="ones")
        nc.vector.memset(ones[:], 1.0)

        # ---- weights: p-major layout; pieces of whole chunks with multi-chunk
        #      contiguous per-partition runs (big descriptors), all on the SP
        #      queue in consumption order ----
        CGRP = [(0, 1), (1, 2), (2, 3), (3, 4), (4, 5), (5, 6), (6, 7)]
        ROW = (NB + 1) * HO  # elems per (p, c)
        weng = [nc.scalar, nc.sync, nc.gpsimd, nc.gpsimd,
                nc.sync, nc.gpsimd, nc.gpsimd]
        wg = []
        off = 0
        for g, (c0, c1) in enumerate(CGRP):
            t = sb.tile([P, c1 - c0, NB + 1, HO], f16, tag=f"wg{g}",
                        name=f"wg{g}")
            run = (c1 - c0) * ROW
            src = bass.AP(tensor=w_d.tensor, offset=off,
                          ap=[[run, P], [1, run]])
            weng[g].dma_start(t[:], src)
            wg.append(t)
            off += P * run

        def wslice(c, j, nh):
            for g, (c0, c1) in enumerate(CGRP):
                if c0 <= c < c1:
                    return wg[g][:, c - c0, j, nh * D2:(nh + 1) * D2]

        c16 = sb.tile([128, 5 * D2 + 128], f16, tag="c16")
        nc.gpsimd.dma_start(c16[:], w1_d)
        w1t = c16[:, 0:5 * D2].rearrange("p (k d) -> p k d", d=D2)
        idt = c16[:, 5 * D2:]
        c32 = sb.tile([128, D2 + H], f32, tag="c32")
        nc.gpsimd.dma_start(c32[:], w2_d)
        w2b = c32[:, 0:D2]
        b2b = c32[:, D2:]
        b1r = sb.tile([1, D2], f16, tag="b1r")
        nc.gpsimd.dma_start(b1r[:], b1_d)

        x = xt[:].rearrange("p c b -> p (c b)")

        def T(tag, dt=f16):
            return sb.tile([P, CH * BC], dt, tag=tag, name=tag)

        # ---- interval index t = floor(5x) via round(5x-0.5); masks; u ----
        ti = T("ti", i32)
        nc.vector.tensor_scalar(ti[:], x, 5.0, -0.5, op0=ALU.mult, op1=ALU.add)
        m = []
        for t in range(5):
            mt = T(f"m{t}")
            nc.vector.tensor_scalar(mt[:], ti[:], t, None, op0=ALU.is_equal)
            m.append(mt)
        tf = T("tf", f32)
        nc.vector.tensor_copy(tf[:], ti[:])
        u = T("u", f32)
        nc.vector.scalar_tensor_tensor(u[:], x, 5.0, tf[:],
                                       op0=ALU.mult, op1=ALU.subtract)

        # ---- local cubics (x6): p0=(1-u)^3, p1=3u^3-6u^2+4=(3u-6)u^2+4,
        #      p2=p1(1-u), p3=u^3 ----
        u2 = T("u2", f32); nc.scalar.activation(u2[:], u[:], AF.Square)
        w_ = T("w_", f32)
        nc.scalar.activation(w_[:], u[:], AF.Copy, bias=1.0, scale=-1.0)
        w2_ = T("w2_", f32); nc.scalar.activation(w2_[:], w_[:], AF.Square)
        a_ = T("a_", f32)
        nc.scalar.activation(a_[:], u[:], AF.Copy, bias=-6.0, scale=3.0)
        b_ = T("b_", f32)
        nc.scalar.activation(b_[:], w_[:], AF.Copy, bias=-6.0, scale=3.0)
        p3h = T("p3h"); nc.vector.tensor_tensor(p3h[:], u2[:], u[:], op=ALU.mult)
        p0h = T("p0h"); nc.vector.tensor_tensor(p0h[:], w2_[:], w_[:], op=ALU.mult)
        p1pre = T("p1pre", f32)
        nc.vector.tensor_tensor(p1pre[:], a_[:], u2[:], op=ALU.mult)
        p1h = T("p1h")
        nc.scalar.activation(p1h[:], p1pre[:], AF.Copy, bias=4.0, scale=1.0)
        p2pre = T("p2pre", f32)
        nc.vector.tensor_tensor(p2pre[:], b_[:], w2_[:], op=ALU.mult)
        p2h = T("p2h")
        nc.scalar.activation(p2h[:], p2pre[:], AF.Copy, bias=4.0, scale=1.0)
        ph = [p0h, p1h, p2h, p3h]

        # ---- features ----
        f_ = []
        for j in range(NB):
            f_.append(sb.tile([P, CH, BC], f16, tag=f"f{j}", name=f"f{j}"))
        fs = sb.tile([P, CH, BC], f16, tag="f8")
        nc.scalar.activation(fs[:].rearrange("p c b -> p (c b)"), x, AF.Silu)
        f_.append(fs)

        psum = [ps.tile([128, D2], f32, tag=f"y{nh}", name=f"y{nh}")
                for nh in range(NH)]

        tmp = T("tmp")
        tmp2 = T("tmp2")
        for j in range(NB):
            terms = [(t, j - t) for t in range(5) if 0 <= j - t <= 3]
            out = f_[j][:].rearrange("p c b -> p (c b)")
            if len(terms) == 1:
                t, r = terms[0]
                nc.vector.tensor_tensor(out, m[t][:], ph[r][:], op=ALU.mult)
            else:
                acc = tmp[:]
                t, r = terms[0]
                nc.vector.tensor_tensor(acc, m[t][:], ph[r][:], op=ALU.mult)
                for k, (t, r) in enumerate(terms[1:]):
                    pr = tmp2[:]
                    nc.vector.tensor_tensor(pr, m[t][:], ph[r][:], op=ALU.mult)
                    dst = out if k == len(terms) - 2 else acc
                    nc.vector.tensor_tensor(dst, acc, pr, op=ALU.add)

        nmm = 0
        NTOT = CH * (NB + 1)
        for g, (c0, c1) in enumerate(CGRP):
            for j in range(NB + 1):
                for c in range(c0, c1):
                    for nh in range(NH):
                        nc.tensor.matmul(
                            psum[nh][:],
                            f_[j][:, c, :],
                            wslice(c, j, nh),
                            start=(nmm == 0),
                            stop=(nmm == NTOT - 1),
                        )
                    nmm += 1

        # ---- tail ----
        h1 = sb.tile([128, HO], f16, tag="h1")
        for nh in range(NH):
            nc.scalar.activation(h1[:, nh * D2:(nh + 1) * D2], psum[nh][:], AF.Tanh)
        h1t = []
        for k in range(5):
            pt = ps.tile([128, 128], f16, tag=f"pt{k}", name=f"pt{k}")
            nc.tensor.transpose(pt[:], h1[:, k * 128:(k + 1) * 128], idt)
            st = sb.tile([128, 128], f16, tag=f"h1t{k}", name=f"h1t{k}")
            nc.vector.tensor_copy(st[:], pt[:])
            h1t.append(st)
        ps2 = ps.tile([128, D2], f32, tag="ps2")
        for k in range(5):
            nc.tensor.matmul(ps2[:], h1t[k][:], w1t[:, k, :],
                             start=(k == 0), stop=False)
        nc.tensor.matmul(ps2[:], ones[:], b1r[:], start=False, stop=True)
        h2 = sb.tile([128, D2], f32, tag="h2")
        nc.scalar.activation(h2[:], ps2[:], AF.Tanh)
        prod = sb.tile([128, D2], f32, tag="prod")
        nc.vector.tensor_tensor(prod[:], h2[:], w2b, op=ALU.mult)
        red = sb.tile([128, H], f32, tag="red")
        nc.vector.tensor_reduce(red[:], prod[:].rearrange("p (h d) -> p h d", d=32),
                                axis=mybir.AxisListType.X, op=ALU.add)
        lg = sb.tile([128, H], f32, tag="lg")
        nc.vector.tensor_tensor(lg[:], red[:], b2b, op=ALU.add)
        nc.sync.dma_start(out_d, lg[:])

    nc.compile()
    _CACHE["nc"] = nc
    return nc


def _prep_inputs(x, coef, scale_base, scale_sp, lmd, W1, b1, W2, b2):
    xf = np.asarray(x, np.float64).reshape(B, I)
    coef = np.asarray(coef, np.float64)
    eff = coef * np.asarray(scale_sp, np.float64)[..., None] \
        * np.asarray(lmd, np.float64)[:, :, None, None] / 6.0
    sbl = np.asarray(scale_base, np.float64) \
        * np.asarray(lmd, np.float64)[:, :, None]
    wbig = np.concatenate([eff, sbl[..., None]], -1)               # (H,I,O,9)
    wp = wbig.reshape(H, CH, P, O, NB + 1).transpose(2, 1, 4, 0, 3) \
        .astype(np.float16)                                        # (P,CH,9,HO)
    wdev = np.ascontiguousarray(wp.transpose(1, 0, 2, 3, 4)).reshape(-1)  # piece-major (per chunk)
    W1 = np.asarray(W1, np.float64)
    w1bd = np.zeros((HO, D2))
    for h in range(H):
        w1bd[h * O:(h + 1) * O, h * 32:(h + 1) * 32] = W1[h]
    w1dev = np.ascontiguousarray(
        w1bd.reshape(5, 128, D2).transpose(1, 0, 2)).astype(np.float16)
    c16 = np.concatenate([w1dev.reshape(128, 5 * D2),
                          np.eye(128, dtype=np.float16)], 1).astype(np.float16)
    b1c = np.asarray(b1, np.float16).reshape(1, D2).copy()
    c32 = np.concatenate([
        np.broadcast_to(np.asarray(W2, np.float32).reshape(D2), (128, D2)),
        np.broadcast_to(np.asarray(b2, np.float32).reshape(H), (128, H))],
        1).astype(np.float32)
    c32 = np.ascontiguousarray(c32)

    in_maps = []
    for core in range(NC):
        xs = xf[core * BC:(core + 1) * BC].T
        xdev = np.ascontiguousarray(
            xs.reshape(CH, P, BC).transpose(1, 0, 2)).astype(np.float32)
        in_maps.append({"x": xdev, "w": wdev, "w1": c16,
                        "b1": b1c, "w2": c32})
    return in_maps


def run(inputs, trace=False, tmpdir=None):
    _install_ntff_hook()
    from concourse.bass_utils import run_bass_kernel_spmd
    nc = _build()
    in_maps = _prep_inputs(**inputs)
    res = run_bass_kernel_spmd(nc, in_maps, core_ids=list(range(NC)),
                               trace=trace, tmpdir=tmpdir)
    out = np.concatenate([r["out"] for r in res.results], 0)
    return out.astype(np.float32), res


def kernel(**inputs):
    out, _ = run(inputs)
    return out
